# revision 8
# baseline (speedup 1.0000x reference)
"""BellmanFord GNN message-passing layer on 8 Trainium2 NeuronCores.

Reference computation (all f32):
    x   = h[src] + edge_rel_emb          # [E, D] gather
    hid = relu(x @ W1 + b1)              # [E, D]
    msg = hid @ W2 + b2                  # [E, D]
    agg = segment_sum(msg, dst, N)       # [N, D]
    out = h + agg

Key algebraic restructuring: since W2 is shared across edges, the second
matmul commutes with the scatter:

    segment_sum(hid @ W2 + b2, dst) = segment_sum(hid, dst) @ W2 + deg*b2

so the per-edge work is only m1 (+relu) and the segment-sum; the W2 matmul
runs once per 128-node window instead of once per edge.

v3 (fp8 + balanced engines):
  - m1 in fp8e4m3 with perf_mode=DoubleRow: the PE virtualizes to 256
    contraction rows, so each 128-edge group's full 256-dim contraction is
    ONE matmul (2/tile instead of 4 bf16; ~109ns each either way).  W1 is
    host-scaled by 16 to keep its entries out of fp8-subnormal range; the
    relu evacuation applies scale=1/16 to undo it.
  - hid is evacuated as fp8 in PAIRED 2-tile ops (1024 cols) alternating
    ScalarE/DVE -- evacuation is PSUM-read-bound (~1ns/col) and is the
    second wall after the PE, so per-op overhead is halved and the load
    split evenly.
  - The scatter's 0/1 S matrix ships pre-built from the host (group slot
    ranges are <=9 wide, padded to 16): one 655KB DMA at kernel start,
    zero per-tile DVE work.  Scatter stationaries are fp8 -> FWL weight
    loads at 4 elem/cycle.
  - A ~3.5us PE warmup burst of dummy matmuls at kernel start trips the
    HAM activity monitor out of its cold 1.2GHz state before real work
    arrives (saves >10us of half-clock execution).
  - xq pair DMAs alternate between the sync and gpsimd rings.

Sharding: edges by destination node range (1250 nodes per core) so each
core owns its output slice outright -- no cross-core reduction.  Host
gathers h[src]+rel and transposes into DoubleRow-interleaved fp8 tiles.
"""

import sys

sys.path.insert(0, "/opt/trn_rl_repo")

import numpy as np
from ml_dtypes import bfloat16, float8_e4m3

import concourse.bass as bass
import concourse.mybir as mybir
import concourse.tile as tile
from concourse import bacc
from concourse.bass_utils import run_bass_kernel_spmd

P = 128
D = 256
N_CORES = 8
ET = 256  # edges per macrotile (2 x P)
WIN = P  # nodes per scatter window
SW = 16  # hosted S width (max scatter group slot range, padded)
F32 = mybir.dt.float32
BF16 = mybir.dt.bfloat16
FP8 = mybir.dt.float8e4
AF = mybir.ActivationFunctionType
DR = mybir.MatmulPerfMode.DoubleRow
W1_SCALE = 16.0  # host premultiplies W1 so fp8 entries stay normal

_CACHE = {}
TRACE = False
TRACE_DIR = "/tmp/ktrace"


def _build_program(n_nodes, tiles_per_window, ranges, has_b1, has_b2):
    """Build the SPMD Bass program. Identical for all 8 cores."""
    npc = n_nodes // N_CORES  # nodes per core
    n_win = len(tiles_per_window)
    n_tiles = int(sum(tiles_per_window))
    n_pairs = (n_tiles + 1) // 2
    npc_pad = n_win * WIN

    nc = bacc.Bacc("TRN2", target_bir_lowering=False, debug=False,
                   num_devices=N_CORES)

    hs_d = nc.dram_tensor("h_slice", [npc_pad, D], F32,
                          kind="ExternalInput").ap()
    # xT tiles, pair-packed, DoubleRow-interleaved fp8:
    # xq[pair, p, g, ko, e] = x[(2*pair+g)*ET + e, ko*P + p]
    xq_d = nc.dram_tensor("xq", [n_pairs, P, 2, 2, ET], FP8,
                          kind="ExternalInput").ap()
    # hosted S: s8[p, t, j, w] = [slot(edge (t,j,p)) - lo_tj == w]
    s8_d = nc.dram_tensor("s8", [P, n_tiles, 2, SW], FP8,
                          kind="ExternalInput").ap()
    w1_d = nc.dram_tensor("w1", [P, 2, D], FP8, kind="ExternalInput").ap()
    w2_d = nc.dram_tensor("w2", [P, 2, D], BF16, kind="ExternalInput").ap()
    b1_d = nc.dram_tensor("b1r", [1, D], BF16, kind="ExternalInput").ap()
    b2_d = nc.dram_tensor("b2r", [1, D], BF16, kind="ExternalInput").ap()
    deg_d = nc.dram_tensor("deg", [1, npc_pad], BF16,
                           kind="ExternalInput").ap()
    out_d = nc.dram_tensor("out", [npc_pad, D], F32, kind="ExternalOutput").ap()

    with tile.TileContext(nc) as tc:
        with (
            tc.tile_pool(name="consts", bufs=1) as cb,
            tc.tile_pool(name="x", bufs=9) as x_pool,
            tc.tile_pool(name="hid", bufs=4) as hid_pool,
            tc.tile_pool(name="HT", bufs=2) as ht_pool,
            tc.tile_pool(name="hw", bufs=3) as h_pool,
            tc.tile_pool(name="outw", bufs=3) as out_pool,
            tc.tile_pool(name="psH", bufs=2, space="PSUM") as ps_h,  # hid
            tc.tile_pool(name="psT", bufs=2, space="PSUM") as ps_t,  # HT acc
            tc.tile_pool(name="psA", bufs=1, space="PSUM") as ps_a,  # agg
        ):
            # ---- warmup fodder (no DMA dependencies) ----
            wdum = cb.tile([P, D], BF16)
            nc.vector.memset(wdum[:], 0.0)

            # consts on the gpsimd DMA ring; w1 first (m1 needs it first),
            # then the one-shot S image
            w1_sb = cb.tile([P, 2, D], FP8)
            nc.gpsimd.dma_start(w1_sb[:], w1_d)
            s_all = cb.tile([P, n_tiles, 2, SW], FP8)
            nc.gpsimd.dma_start(s_all[:], s8_d)
            w2_sb = cb.tile([P, 2, D], BF16)
            nc.gpsimd.dma_start(w2_sb[:], w2_d)
            b1_sb = cb.tile([1, D], BF16)
            nc.gpsimd.dma_start(b1_sb[:], b1_d)
            b2_sb = cb.tile([1, D], BF16)
            nc.gpsimd.dma_start(b2_sb[:], b2_d)
            deg_sb = cb.tile([1, npc_pad], BF16)
            nc.gpsimd.dma_start(deg_sb[:], deg_d)
            if has_b1:
                ones_sb = cb.tile([1, P], BF16)
                nc.vector.memset(ones_sb[:], 1.0)

            # ---- PE warmup burst: ~3.5us of dummy matmuls at cold clock
            # trips HAM to 2.4GHz before the first real m1.  Writes go to
            # the agg bank (psA) which the real pipeline only claims from
            # superstep ~26, so no clash with the hid pair banks. ----
            warm_ps = ps_a.tile([P, D], F32, name="agg_ps")
            for i in range(16):
                nc.tensor.matmul(warm_ps[:], lhsT=wdum[:, 0:P],
                                 rhs=wdum[:], start=True, stop=True)

            # ---- software-pipelined emission ----
            tile_win = []
            for w in range(n_win):
                tile_win += [w] * tiles_per_window[w]
            win_first = {}
            win_last = {}
            for ti, w in enumerate(tile_win):
                win_first.setdefault(w, ti)
                win_last[w] = ti
            # last non-empty (tile, j) scatter group per window, for the
            # accumulation-group stop flag
            win_lastg = {}
            for ti, w in enumerate(tile_win):
                for j in range(2):
                    if ranges[2 * ti + j][0] < ranges[2 * ti + j][1]:
                        win_lastg[w] = (ti, j)
            T = n_tiles
            st = {}  # per-tile live tiles
            pair_state = {}
            win_state = {}

            def s_dma(t):
                if t % 2 != 0:
                    return
                x_sb = x_pool.tile([P, 2, 2, ET], FP8, name="x_sb")
                ring = nc.sync if (t // 2) % 2 == 0 else nc.gpsimd
                ring.dma_start(x_sb[:], xq_d[t // 2])
                st[t] = {"x": x_sb, "g": 0}
                if t + 1 < T:
                    st[t + 1] = {"x": x_sb, "g": 1}

            def s_m1(t):
                x_sb = st[t]["x"]
                g = st[t]["g"]
                if t % 2 == 0:
                    hid_ps = ps_h.tile([P, 2, 2, D], F32, name="hid_ps")
                    pair_state[t // 2] = hid_ps
                hid_ps = pair_state[t // 2]
                a = t % 2
                for j in range(2):
                    # one DoubleRow matmul: full 256-dim contraction
                    nc.tensor.matmul(
                        hid_ps[:, a, j],
                        lhsT=x_sb[:, g, :, j * P:(j + 1) * P],
                        rhs=w1_sb[:],
                        start=True,
                        stop=not has_b1,
                        perf_mode=DR,
                    )
                    if has_b1:
                        nc.tensor.matmul(
                            hid_ps[:, a, j],
                            lhsT=ones_sb[:],
                            rhs=b1_sb[:],
                            start=False,
                            stop=True,
                        )

            def s_relu(t):
                # evacuate the (t-1, t) pair in one 1024-col op; scale=1/16
                # undoes the host-side W1 premultiply
                if t % 2 == 0 and t != T - 1:
                    return
                pr = t // 2
                hid_ps = pair_state.pop(pr)
                hid_sb = hid_pool.tile([P, 2, 2, D], FP8, name="hid_sb")
                if pr % 2 == 0:
                    nc.scalar.activation(hid_sb[:], hid_ps[:], AF.Relu,
                                         scale=1.0 / W1_SCALE)
                else:
                    nc.vector.tensor_scalar(
                        out=hid_sb[:], in0=hid_ps[:],
                        scalar1=1.0 / W1_SCALE, scalar2=0.0,
                        op0=mybir.AluOpType.mult,
                        op1=mybir.AluOpType.max)
                st[t]["hidp"] = hid_sb
                if t % 2 == 1:
                    st[t - 1]["hidp"] = hid_sb

            def s_sc(t):
                w = tile_win[t]
                if t == win_first[w]:
                    ht_ps = ps_t.tile([P, 2, P], F32, name="ht_ps")
                    nc.vector.memset(ht_ps[:], 0.0)
                    h_sb = h_pool.tile([P, D], F32, name="h_sb")
                    nc.gpsimd.dma_start(h_sb[:],
                                        hs_d[w * WIN:(w + 1) * WIN])
                    win_state[w] = (ht_ps, h_sb)
                ht_ps, _ = win_state[w]
                a = t % 2
                hid_sb = st[t]["hidp"]
                for j in range(2):
                    lo, hi = ranges[2 * t + j]
                    if lo >= hi:
                        continue
                    for dc in range(2):
                        nc.tensor.matmul(
                            ht_ps[:, dc, lo:hi],
                            lhsT=hid_sb[:, a, j, dc * P:(dc + 1) * P],
                            rhs=s_all[:, t, j, 0:hi - lo],
                            start=False,
                            stop=(win_lastg[w] == (t, j)),
                            skip_group_check=True,
                        )
                del st[t]

            def s_htcopy(t):
                w = tile_win[t]
                if t != win_last[w]:
                    return
                ht_ps, h_sb = win_state[w]
                ht_sb = ht_pool.tile([P, 2, P], BF16, name="ht_sb")
                nc.scalar.copy(ht_sb[:], ht_ps[:])
                win_state[w] = (ht_sb, h_sb)

            def s_fin(t):
                w = tile_win[t]
                if t != win_last[w]:
                    return
                ht_sb, h_sb = win_state[w]
                agg_ps = ps_a.tile([P, D], F32, name="agg_ps")
                for dc in range(2):
                    nc.tensor.matmul(
                        agg_ps[:],
                        lhsT=ht_sb[:, dc],
                        rhs=w2_sb[:, dc],
                        start=(dc == 0),
                        stop=(dc == 1 and not has_b2),
                    )
                if has_b2:
                    nc.tensor.matmul(
                        agg_ps[:],
                        lhsT=deg_sb[:, w * WIN:(w + 1) * WIN],
                        rhs=b2_sb[:],
                        start=False,
                        stop=True,
                    )
                out_sb = out_pool.tile([P, D], F32, name="out_sb")
                nc.vector.tensor_add(out_sb[:], agg_ps[:], h_sb[:])
                nc.sync.dma_start(out_d[w * WIN:(w + 1) * WIN], out_sb[:])
                del win_state[w]

            # (stage, lag): emission order within a step puts the oldest
            # tile's PE work first so the PE never head-of-line blocks on
            # freshly produced inputs.  PE *stationary* operands (m1's xT,
            # the scatter's hid, m2's HT) must be >= 2 super-steps old when
            # loaded: the PE silicon pulls LDWEIGHTS ahead of in-flight
            # MATMULs, bypassing the semaphore that gates the MATMUL, so a
            # freshly written weight tile can be read stale.
            stages = [(s_fin, 11), (s_htcopy, 9), (s_sc, 7), (s_relu, 4),
                      (s_m1, 3), (s_dma, -5)]
            L = 1 + max(lag for _, lag in stages)
            lag0 = min(lag for _, lag in stages)
            for i in range(lag0, T + L - 1):
                # ballast: keep the PE dense through the pipeline-fill phase
                # so the HAM activity monitor never re-throttles the clock
                if 0 <= i < 22:
                    for _ in range(2):
                        nc.tensor.matmul(warm_ps[:], lhsT=wdum[:, 0:P],
                                         rhs=wdum[:], start=True, stop=True)
                for fn, lag in stages:
                    t_ = i - lag
                    if 0 <= t_ < T:
                        fn(t_)

    nc.compile()
    return nc


def _prepare_shards(h, src, dst, rel, n_nodes):
    """Shard edges by destination range; balance nodes across the per-core
    128-node scatter windows by degree (LPT) so every window's edge count,
    and therefore the shared per-window tile schedule, is near-uniform.

    Returns per-core input arrays, the shared tiles_per_window schedule and
    per-core node->device-row maps for unpermuting the output."""
    npc = n_nodes // N_CORES
    n_win = (npc + WIN - 1) // WIN
    npc_pad = n_win * WIN

    cores = []
    counts = np.zeros((N_CORES, n_win), dtype=np.int64)
    for c in range(N_CORES):
        lo = c * npc
        mask = (dst >= lo) & (dst < lo + npc)
        idx = np.nonzero(mask)[0]
        d_c = (dst[idx] - lo).astype(np.int64)
        deg_cnt = np.bincount(d_c, minlength=npc)
        # LPT: nodes by degree desc -> least-loaded window with free slots
        node_win = np.zeros(npc, dtype=np.int64)
        node_slot = np.zeros(npc, dtype=np.int64)
        win_load = np.zeros(n_win, dtype=np.int64)
        win_fill = np.zeros(n_win, dtype=np.int64)
        for nd in np.argsort(-deg_cnt, kind="stable"):
            open_w = np.nonzero(win_fill < WIN)[0]
            wsel = open_w[np.argmin(win_load[open_w])]
            node_win[nd] = wsel
            node_slot[nd] = win_fill[wsel]
            win_fill[wsel] += 1
            win_load[wsel] += deg_cnt[nd]
        counts[c] = win_load
        cores.append((idx, d_c, node_win, node_slot, deg_cnt))

    tiles_per_window = [
        max(1, int(-(-counts[:, w].max() // ET))) for w in range(n_win)
    ]
    n_tiles = int(sum(tiles_per_window))
    n_pairs = (n_tiles + 1) // 2
    starts = np.concatenate([[0], np.cumsum(tiles_per_window)])

    # Per-(tile, j-group) node-slot ranges: edges are sorted by slot within
    # a window, so each 128-edge group touches only a short contiguous slot
    # range.  The SPMD program is shared, so take the union across cores.
    rng_lo = np.full((n_tiles, 2), P, dtype=np.int64)
    rng_hi = np.zeros((n_tiles, 2), dtype=np.int64)

    percore = []
    node_rows_all = []
    for c in range(N_CORES):
        idx, d_c, node_win, node_slot, deg_cnt = cores[c]
        e_win = node_win[d_c]
        order = np.argsort(e_win, kind="stable")
        idx = idx[order]
        d_c = d_c[order]
        e_win = e_win[order]

        src_pad = np.zeros(n_tiles * ET, dtype=np.int64)
        dloc_pad = np.full(n_tiles * ET, -1.0, dtype=np.float32)
        rel_rows = np.zeros((n_tiles * ET, D), dtype=np.float32)
        real = np.zeros(n_tiles * ET, dtype=bool)
        bounds = np.searchsorted(e_win, np.arange(n_win + 1))
        for w in range(n_win):
            a, b = bounds[w], bounds[w + 1]
            k = b - a
            off = int(starts[w]) * ET
            # sort this window's edges by slot so tile j-groups cover
            # contiguous slot ranges
            slots = node_slot[d_c[a:b]]
            sord = np.argsort(slots, kind="stable")
            src_pad[off:off + k] = src[idx[a:b]][sord]
            dloc_pad[off:off + k] = slots[sord].astype(np.float32)
            rel_rows[off:off + k] = rel[idx[a:b]][sord]
            real[off:off + k] = True
            ss = slots[sord]
            for t in range(tiles_per_window[w]):
                gt = int(starts[w]) + t
                for j in range(2):
                    g0 = t * ET + j * P
                    g1 = min(g0 + P, k)
                    if g0 >= k:
                        continue
                    rng_lo[gt, j] = min(rng_lo[gt, j], int(ss[g0]))
                    rng_hi[gt, j] = max(rng_hi[gt, j], int(ss[g1 - 1]) + 1)

        x_pad = h[src_pad].astype(np.float32)
        x_pad += rel_rows
        x_pad[~real] = 0.0
        x8 = x_pad.astype(float8_e4m3)
        if n_pairs * 2 * ET > x8.shape[0]:
            x8 = np.concatenate(
                [x8, np.zeros((n_pairs * 2 * ET - x8.shape[0], D),
                              dtype=float8_e4m3)], axis=0)
        # [pair, p, g, ko, e]: DoubleRow-interleaved (k = ko*128 + ki)
        xq = np.ascontiguousarray(
            x8.reshape(n_pairs, 2, ET, 2, P).transpose(0, 4, 1, 3, 2))

        node_rows = node_win * WIN + node_slot  # device row of local node
        h_slice = np.zeros((npc_pad, D), dtype=np.float32)
        h_slice[node_rows] = h[c * npc:(c + 1) * npc]
        deg = np.zeros(npc_pad, dtype=np.float32)
        deg[node_rows] = deg_cnt

        percore.append({
            "h_slice": h_slice,
            "xq": xq,
            "dloc_pad": dloc_pad,
            "deg": deg.astype(bfloat16).reshape(1, npc_pad),
        })
        node_rows_all.append(node_rows)

    ranges = tuple(
        (int(min(rng_lo[t, j], rng_hi[t, j])), int(rng_hi[t, j]))
        for t in range(n_tiles) for j in range(2))
    wmax = max(r[1] - r[0] for r in
               [(ranges[g][0], ranges[g][1]) for g in range(2 * n_tiles)])
    assert wmax <= SW, f"scatter group width {wmax} exceeds hosted S {SW}"

    # hosted S image: s8[p, t, j, w] = [slot(edge (t,j,p)) - lo_tj == w]
    lo_per_group = np.array([ranges[g][0] for g in range(2 * n_tiles)],
                            dtype=np.float32)
    in_maps = []
    for c in range(N_CORES):
        m = percore[c]
        dloc = m.pop("dloc_pad").reshape(2 * n_tiles, P)
        rel_slot = dloc - lo_per_group[:, None]  # [2*n_tiles, P]
        s8 = np.zeros((2 * n_tiles, P, SW), dtype=float8_e4m3)
        gi, pi = np.nonzero((rel_slot >= 0) & (rel_slot < SW))
        s8[gi, pi, rel_slot[gi, pi].astype(np.int64)] = 1.0
        # -> [p, t, j, w]
        m["s8"] = np.ascontiguousarray(
            s8.reshape(n_tiles, 2, P, SW).transpose(2, 0, 1, 3))
        in_maps.append(m)

    return in_maps, tiles_per_window, ranges, npc, n_win, node_rows_all


def kernel(h, edge_index, edge_rel_emb, W1, b1, W2, b2, num_nodes):
    h = np.ascontiguousarray(h, dtype=np.float32)
    rel = np.ascontiguousarray(edge_rel_emb, dtype=np.float32)
    W1 = np.ascontiguousarray(W1, dtype=np.float32)
    W2 = np.ascontiguousarray(W2, dtype=np.float32)
    b1 = np.ascontiguousarray(b1, dtype=np.float32)
    b2 = np.ascontiguousarray(b2, dtype=np.float32)
    n_nodes = int(num_nodes)
    src = np.asarray(edge_index[0]).astype(np.int64)
    dst = np.asarray(edge_index[1]).astype(np.int64)
    assert n_nodes % N_CORES == 0
    assert h.shape == (n_nodes, D)

    (in_maps, tiles_per_window, ranges, npc, n_win,
     node_rows_all) = _prepare_shards(h, src, dst, rel, n_nodes)

    has_b1 = bool(np.any(b1))
    has_b2 = bool(np.any(b2))
    key = (n_nodes, tuple(tiles_per_window), ranges, has_b1, has_b2)
    if key not in _CACHE:
        _CACHE[key] = _build_program(n_nodes, tiles_per_window, ranges,
                                     has_b1, has_b2)
    nc = _CACHE[key]

    w1_r = np.ascontiguousarray(
        (W1 * W1_SCALE).reshape(2, P, D).transpose(1, 0, 2)).astype(
            float8_e4m3)
    w2_r = np.ascontiguousarray(
        W2.reshape(2, P, D).transpose(1, 0, 2)).astype(bfloat16)
    for m in in_maps:
        m["w1"] = w1_r
        m["w2"] = w2_r
        m["b1r"] = (b1 * W1_SCALE).reshape(1, D).astype(bfloat16)
        m["b2r"] = b2.reshape(1, D).astype(bfloat16)

    trace_kwargs = {}
    if TRACE:
        trace_kwargs = dict(trace=True, tmpdir=TRACE_DIR,
                            trace_cores=list(range(N_CORES)))
    res = run_bass_kernel_spmd(nc, in_maps, core_ids=list(range(N_CORES)),
                               **trace_kwargs)
    out = np.concatenate(
        [np.asarray(res.results[c]["out"])[node_rows_all[c]]
         for c in range(N_CORES)], axis=0)

    # stash for test harnesses
    kernel.last_results = res
    return out.astype(np.float32)


# revision 11
# speedup vs baseline: 1.1324x; 1.1324x over previous
"""BellmanFord GNN message-passing layer on 8 Trainium2 NeuronCores.

Reference computation (all f32):
    x   = h[src] + edge_rel_emb          # [E, D] gather
    hid = relu(x @ W1 + b1)              # [E, D]
    msg = hid @ W2 + b2                  # [E, D]
    agg = segment_sum(msg, dst, N)       # [N, D]
    out = h + agg

Key algebraic restructuring: since W2 is shared across edges, the second
matmul commutes with the scatter:

    segment_sum(hid @ W2 + b2, dst) = segment_sum(hid, dst) @ W2 + deg*b2

so the per-edge work is only m1 (+relu) and the segment-sum; the W2 matmul
runs once per 128-node window instead of once per edge.

v3 (fp8 + balanced engines):
  - m1 in fp8e4m3 with perf_mode=DoubleRow: the PE virtualizes to 256
    contraction rows, so each 128-edge group's full 256-dim contraction is
    ONE matmul (2/tile instead of 4 bf16; ~109ns each either way).  W1 is
    host-scaled by 16 to keep its entries out of fp8-subnormal range; the
    relu evacuation applies scale=1/16 to undo it.
  - hid is evacuated as fp8 in PAIRED 2-tile ops (1024 cols) alternating
    ScalarE/DVE -- evacuation is PSUM-read-bound (~1ns/col) and is the
    second wall after the PE, so per-op overhead is halved and the load
    split evenly.
  - The scatter's 0/1 S matrix ships pre-built from the host (group slot
    ranges are <=9 wide, padded to 16): one 655KB DMA at kernel start,
    zero per-tile DVE work.  Scatter stationaries are fp8 -> FWL weight
    loads at 4 elem/cycle.
  - A ~3.5us PE warmup burst of dummy matmuls at kernel start trips the
    HAM activity monitor out of its cold 1.2GHz state before real work
    arrives (saves >10us of half-clock execution).
  - xq pair DMAs alternate between the sync and gpsimd rings.

Sharding: edges by destination node range (1250 nodes per core) so each
core owns its output slice outright -- no cross-core reduction.  Host
gathers h[src]+rel and transposes into DoubleRow-interleaved fp8 tiles.
"""

import sys

sys.path.insert(0, "/opt/trn_rl_repo")

import numpy as np
from ml_dtypes import bfloat16, float8_e4m3

import concourse.bass as bass
import concourse.mybir as mybir
import concourse.tile as tile
from concourse import bacc
from concourse.bass_utils import run_bass_kernel_spmd

P = 128
D = 256
N_CORES = 8
ET = 256  # edges per macrotile (2 x P)
WIN = P  # nodes per scatter window
SW = 16  # hosted S width (max scatter group slot range, padded)
F32 = mybir.dt.float32
BF16 = mybir.dt.bfloat16
FP8 = mybir.dt.float8e4
AF = mybir.ActivationFunctionType
DR = mybir.MatmulPerfMode.DoubleRow
W1_SCALE = 16.0  # host premultiplies W1 so fp8 entries stay normal

_CACHE = {}
TRACE = False
TRACE_DIR = "/tmp/ktrace"


def _build_program(n_nodes, tiles_per_window, ranges, has_b1, has_b2):
    """Build the SPMD Bass program. Identical for all 8 cores."""
    npc = n_nodes // N_CORES  # nodes per core
    n_win = len(tiles_per_window)
    n_tiles = int(sum(tiles_per_window))
    n_pairs = (n_tiles + 1) // 2
    npc_pad = n_win * WIN

    nc = bacc.Bacc("TRN2", target_bir_lowering=False, debug=False,
                   num_devices=N_CORES)

    hs_d = nc.dram_tensor("h_slice", [npc_pad, D], F32,
                          kind="ExternalInput").ap()
    # xT tiles, pair-packed, DoubleRow-interleaved fp8:
    # xq[pair, p, g, ko, e] = x[(2*pair+g)*ET + e, ko*P + p]
    xq_d = nc.dram_tensor("xq", [n_pairs, P, 2, 2, ET], FP8,
                          kind="ExternalInput").ap()
    # hosted S: s8[p, t, j, w] = [slot(edge (t,j,p)) - lo_tj == w]
    s8_d = nc.dram_tensor("s8", [P, n_tiles, 2, SW], FP8,
                          kind="ExternalInput").ap()
    w1_d = nc.dram_tensor("w1", [P, 2, D], FP8, kind="ExternalInput").ap()
    w2_d = nc.dram_tensor("w2", [P, 2, D], BF16, kind="ExternalInput").ap()
    b1_d = nc.dram_tensor("b1r", [1, D], BF16, kind="ExternalInput").ap()
    b2_d = nc.dram_tensor("b2r", [1, D], BF16, kind="ExternalInput").ap()
    deg_d = nc.dram_tensor("deg", [1, npc_pad], BF16,
                           kind="ExternalInput").ap()
    out_d = nc.dram_tensor("out", [npc_pad, D], F32, kind="ExternalOutput").ap()

    with tile.TileContext(nc) as tc:
        with (
            tc.tile_pool(name="consts", bufs=1) as cb,
            tc.tile_pool(name="x", bufs=9) as x_pool,
            tc.tile_pool(name="hid", bufs=4) as hid_pool,
            tc.tile_pool(name="HT", bufs=2) as ht_pool,
            tc.tile_pool(name="hw", bufs=3) as h_pool,
            tc.tile_pool(name="outw", bufs=3) as out_pool,
            tc.tile_pool(name="psH", bufs=2, space="PSUM") as ps_h,  # hid
            tc.tile_pool(name="psT", bufs=2, space="PSUM") as ps_t,  # HT acc
            tc.tile_pool(name="psA", bufs=1, space="PSUM") as ps_a,  # agg
        ):
            # ---- warmup fodder (no DMA dependencies) ----
            wdum = cb.tile([P, D], BF16)
            nc.vector.memset(wdum[:], 0.0)

            # consts on the gpsimd DMA ring; w1 first (m1 needs it first),
            # then the one-shot S image
            w1_sb = cb.tile([P, 2, D], FP8)
            nc.gpsimd.dma_start(w1_sb[:], w1_d)
            s_all = cb.tile([P, n_tiles, 2, SW], FP8)
            nc.gpsimd.dma_start(s_all[:], s8_d)
            w2_sb = cb.tile([P, 2, D], BF16)
            nc.gpsimd.dma_start(w2_sb[:], w2_d)
            b1_sb = cb.tile([1, D], BF16)
            nc.gpsimd.dma_start(b1_sb[:], b1_d)
            b2_sb = cb.tile([1, D], BF16)
            nc.gpsimd.dma_start(b2_sb[:], b2_d)
            deg_sb = cb.tile([1, npc_pad], BF16)
            nc.gpsimd.dma_start(deg_sb[:], deg_d)
            if has_b1:
                ones_sb = cb.tile([1, P], BF16)
                nc.vector.memset(ones_sb[:], 1.0)

            # ---- PE warmup burst: ~3.5us of dummy matmuls at cold clock
            # trips HAM to 2.4GHz before the first real m1.  Writes go to
            # the agg bank (psA) which the real pipeline only claims from
            # superstep ~26, so no clash with the hid pair banks. ----
            warm_ps = ps_a.tile([P, D], F32, name="agg_ps")
            for i in range(16):
                nc.tensor.matmul(warm_ps[:], lhsT=wdum[:, 0:P],
                                 rhs=wdum[:], start=True, stop=True)

            # ---- software-pipelined emission ----
            tile_win = []
            for w in range(n_win):
                tile_win += [w] * tiles_per_window[w]
            win_first = {}
            win_last = {}
            for ti, w in enumerate(tile_win):
                win_first.setdefault(w, ti)
                win_last[w] = ti
            # last non-empty (tile, j) scatter group per window, for the
            # accumulation-group stop flag
            win_lastg = {}
            for ti, w in enumerate(tile_win):
                for j in range(2):
                    if ranges[2 * ti + j][0] < ranges[2 * ti + j][1]:
                        win_lastg[w] = (ti, j)
            T = n_tiles
            st = {}  # per-tile live tiles
            pair_state = {}
            win_state = {}

            def s_dma(t):
                if t % 2 != 0:
                    return
                x_sb = x_pool.tile([P, 2, 2, ET], FP8, name="x_sb")
                ring = nc.sync if (t // 2) % 2 == 0 else nc.gpsimd
                ring.dma_start(x_sb[:], xq_d[t // 2])
                st[t] = {"x": x_sb, "g": 0}
                if t + 1 < T:
                    st[t + 1] = {"x": x_sb, "g": 1}

            def s_m1(t):
                x_sb = st[t]["x"]
                g = st[t]["g"]
                if t % 2 == 0:
                    hid_ps = ps_h.tile([P, 2, 2, D], F32, name="hid_ps")
                    pair_state[t // 2] = hid_ps
                hid_ps = pair_state[t // 2]
                a = t % 2
                for j in range(2):
                    # one DoubleRow matmul: full 256-dim contraction
                    nc.tensor.matmul(
                        hid_ps[:, a, j],
                        lhsT=x_sb[:, g, :, j * P:(j + 1) * P],
                        rhs=w1_sb[:],
                        start=True,
                        stop=not has_b1,
                        perf_mode=DR,
                    )
                    if has_b1:
                        nc.tensor.matmul(
                            hid_ps[:, a, j],
                            lhsT=ones_sb[:],
                            rhs=b1_sb[:],
                            start=False,
                            stop=True,
                        )

            def s_relu(t):
                # evacuate the (t-1, t) pair in one 1024-col op; scale=1/16
                # undoes the host-side W1 premultiply
                if t % 2 == 0 and t != T - 1:
                    return
                pr = t // 2
                hid_ps = pair_state.pop(pr)
                hid_sb = hid_pool.tile([P, 2, 2, D], FP8, name="hid_sb")
                if pr % 2 == 0:
                    nc.scalar.activation(hid_sb[:], hid_ps[:], AF.Relu,
                                         scale=1.0 / W1_SCALE)
                else:
                    nc.vector.tensor_scalar(
                        out=hid_sb[:], in0=hid_ps[:],
                        scalar1=1.0 / W1_SCALE, scalar2=0.0,
                        op0=mybir.AluOpType.mult,
                        op1=mybir.AluOpType.max)
                st[t]["hidp"] = hid_sb
                if t % 2 == 1:
                    st[t - 1]["hidp"] = hid_sb

            def s_wopen(t):
                w = tile_win[t]
                if t != win_first[w]:
                    return
                ht_ps = ps_t.tile([P, 2, P], F32, name="ht_ps")
                nc.vector.memset(ht_ps[:], 0.0)
                h_sb = h_pool.tile([P, D], F32, name="h_sb")
                nc.gpsimd.dma_start(h_sb[:], hs_d[w * WIN:(w + 1) * WIN])
                win_state[w] = (ht_ps, h_sb)

            def s_sc(t):
                w = tile_win[t]
                ht_ps, _ = win_state[w]
                a = t % 2
                hid_sb = st[t]["hidp"]
                for j in range(2):
                    lo, hi = ranges[2 * t + j]
                    if lo >= hi:
                        continue
                    for dc in range(2):
                        nc.tensor.matmul(
                            ht_ps[:, dc, lo:hi],
                            lhsT=hid_sb[:, a, j, dc * P:(dc + 1) * P],
                            rhs=s_all[:, t, j, 0:hi - lo],
                            start=False,
                            stop=(win_lastg[w] == (t, j)),
                            skip_group_check=True,
                        )
                del st[t]

            def s_htcopy(t):
                w = tile_win[t]
                if t != win_last[w]:
                    return
                ht_ps, h_sb = win_state[w]
                ht_sb = ht_pool.tile([P, 2, P], BF16, name="ht_sb")
                # alternate engines so the copy doesn't queue behind this
                # window's relu ops on one engine
                if w % 2 == 0:
                    nc.scalar.copy(ht_sb[:], ht_ps[:])
                else:
                    nc.vector.tensor_copy(ht_sb[:], ht_ps[:])
                win_state[w] = (ht_sb, h_sb)

            def s_fin(t):
                w = tile_win[t]
                if t != win_last[w]:
                    return
                ht_sb, h_sb = win_state[w]
                agg_ps = ps_a.tile([P, D], F32, name="agg_ps")
                for dc in range(2):
                    nc.tensor.matmul(
                        agg_ps[:],
                        lhsT=ht_sb[:, dc],
                        rhs=w2_sb[:, dc],
                        start=(dc == 0),
                        stop=(dc == 1 and not has_b2),
                    )
                if has_b2:
                    nc.tensor.matmul(
                        agg_ps[:],
                        lhsT=deg_sb[:, w * WIN:(w + 1) * WIN],
                        rhs=b2_sb[:],
                        start=False,
                        stop=True,
                    )
                out_sb = out_pool.tile([P, D], F32, name="out_sb")
                nc.vector.tensor_add(out_sb[:], agg_ps[:], h_sb[:])
                nc.sync.dma_start(out_d[w * WIN:(w + 1) * WIN], out_sb[:])
                del win_state[w]

            # (stage, lag): emission order within a step puts the oldest
            # tile's PE work first so the PE never head-of-line blocks on
            # freshly produced inputs.  PE *stationary* operands (m1's xT,
            # the scatter's hid, m2's HT) must be >= 2 super-steps old when
            # loaded: the PE silicon pulls LDWEIGHTS ahead of in-flight
            # MATMULs, bypassing the semaphore that gates the MATMUL, so a
            # freshly written weight tile can be read stale.
            stages = [(s_fin, 13), (s_htcopy, 9), (s_sc, 7), (s_wopen, 5),
                      (s_relu, 4), (s_m1, 3), (s_dma, -5)]
            L = 1 + max(lag for _, lag in stages)
            lag0 = min(lag for _, lag in stages)
            # first s_fin superstep: ballast into the psA bank must stop
            # before the real agg claims it
            fin0 = min(t for t in range(T)
                       if t == win_last[tile_win[t]]) + 13
            for i in range(lag0, T + L - 1):
                # ballast: keep the PE dense through the pipeline-fill phase
                # so the HAM activity monitor never re-throttles the clock
                if 0 <= i < fin0 - 2:
                    nc.tensor.matmul(warm_ps[:], lhsT=wdum[:, 0:P],
                                     rhs=wdum[:], start=True, stop=True)
                for fn, lag in stages:
                    t_ = i - lag
                    if 0 <= t_ < T:
                        fn(t_)

    nc.compile()
    return nc


def _prepare_shards(h, src, dst, rel, n_nodes):
    """Shard edges by destination range; balance nodes across the per-core
    128-node scatter windows by degree (LPT) so every window's edge count,
    and therefore the shared per-window tile schedule, is near-uniform.

    Returns per-core input arrays, the shared tiles_per_window schedule and
    per-core node->device-row maps for unpermuting the output."""
    npc = n_nodes // N_CORES
    n_win = (npc + WIN - 1) // WIN
    npc_pad = n_win * WIN

    cores = []
    counts = np.zeros((N_CORES, n_win), dtype=np.int64)
    for c in range(N_CORES):
        lo = c * npc
        mask = (dst >= lo) & (dst < lo + npc)
        idx = np.nonzero(mask)[0]
        d_c = (dst[idx] - lo).astype(np.int64)
        deg_cnt = np.bincount(d_c, minlength=npc)
        # LPT: nodes by degree desc -> least-loaded window with free slots
        node_win = np.zeros(npc, dtype=np.int64)
        node_slot = np.zeros(npc, dtype=np.int64)
        win_load = np.zeros(n_win, dtype=np.int64)
        win_fill = np.zeros(n_win, dtype=np.int64)
        for nd in np.argsort(-deg_cnt, kind="stable"):
            open_w = np.nonzero(win_fill < WIN)[0]
            wsel = open_w[np.argmin(win_load[open_w])]
            node_win[nd] = wsel
            node_slot[nd] = win_fill[wsel]
            win_fill[wsel] += 1
            win_load[wsel] += deg_cnt[nd]
        counts[c] = win_load
        cores.append((idx, d_c, node_win, node_slot, deg_cnt))

    tiles_per_window = [
        max(1, int(-(-counts[:, w].max() // ET))) for w in range(n_win)
    ]
    n_tiles = int(sum(tiles_per_window))
    n_pairs = (n_tiles + 1) // 2
    starts = np.concatenate([[0], np.cumsum(tiles_per_window)])

    # Per-(tile, j-group) node-slot ranges: edges are sorted by slot within
    # a window, so each 128-edge group touches only a short contiguous slot
    # range.  The SPMD program is shared, so take the union across cores.
    rng_lo = np.full((n_tiles, 2), P, dtype=np.int64)
    rng_hi = np.zeros((n_tiles, 2), dtype=np.int64)

    percore = []
    node_rows_all = []
    for c in range(N_CORES):
        idx, d_c, node_win, node_slot, deg_cnt = cores[c]
        e_win = node_win[d_c]
        order = np.argsort(e_win, kind="stable")
        idx = idx[order]
        d_c = d_c[order]
        e_win = e_win[order]

        src_pad = np.zeros(n_tiles * ET, dtype=np.int64)
        dloc_pad = np.full(n_tiles * ET, -1.0, dtype=np.float32)
        rel_rows = np.zeros((n_tiles * ET, D), dtype=np.float32)
        real = np.zeros(n_tiles * ET, dtype=bool)
        bounds = np.searchsorted(e_win, np.arange(n_win + 1))
        for w in range(n_win):
            a, b = bounds[w], bounds[w + 1]
            k = b - a
            off = int(starts[w]) * ET
            # sort this window's edges by slot so tile j-groups cover
            # contiguous slot ranges
            slots = node_slot[d_c[a:b]]
            sord = np.argsort(slots, kind="stable")
            src_pad[off:off + k] = src[idx[a:b]][sord]
            dloc_pad[off:off + k] = slots[sord].astype(np.float32)
            rel_rows[off:off + k] = rel[idx[a:b]][sord]
            real[off:off + k] = True
            ss = slots[sord]
            for t in range(tiles_per_window[w]):
                gt = int(starts[w]) + t
                for j in range(2):
                    g0 = t * ET + j * P
                    g1 = min(g0 + P, k)
                    if g0 >= k:
                        continue
                    rng_lo[gt, j] = min(rng_lo[gt, j], int(ss[g0]))
                    rng_hi[gt, j] = max(rng_hi[gt, j], int(ss[g1 - 1]) + 1)

        x_pad = h[src_pad].astype(np.float32)
        x_pad += rel_rows
        x_pad[~real] = 0.0
        x8 = x_pad.astype(float8_e4m3)
        if n_pairs * 2 * ET > x8.shape[0]:
            x8 = np.concatenate(
                [x8, np.zeros((n_pairs * 2 * ET - x8.shape[0], D),
                              dtype=float8_e4m3)], axis=0)
        # [pair, p, g, ko, e]: DoubleRow-interleaved (k = ko*128 + ki)
        xq = np.ascontiguousarray(
            x8.reshape(n_pairs, 2, ET, 2, P).transpose(0, 4, 1, 3, 2))

        node_rows = node_win * WIN + node_slot  # device row of local node
        h_slice = np.zeros((npc_pad, D), dtype=np.float32)
        h_slice[node_rows] = h[c * npc:(c + 1) * npc]
        deg = np.zeros(npc_pad, dtype=np.float32)
        deg[node_rows] = deg_cnt

        percore.append({
            "h_slice": h_slice,
            "xq": xq,
            "dloc_pad": dloc_pad,
            "deg": deg.astype(bfloat16).reshape(1, npc_pad),
        })
        node_rows_all.append(node_rows)

    ranges = tuple(
        (int(min(rng_lo[t, j], rng_hi[t, j])), int(rng_hi[t, j]))
        for t in range(n_tiles) for j in range(2))
    wmax = max(r[1] - r[0] for r in
               [(ranges[g][0], ranges[g][1]) for g in range(2 * n_tiles)])
    assert wmax <= SW, f"scatter group width {wmax} exceeds hosted S {SW}"

    # hosted S image: s8[p, t, j, w] = [slot(edge (t,j,p)) - lo_tj == w]
    lo_per_group = np.array([ranges[g][0] for g in range(2 * n_tiles)],
                            dtype=np.float32)
    in_maps = []
    for c in range(N_CORES):
        m = percore[c]
        dloc = m.pop("dloc_pad").reshape(2 * n_tiles, P)
        rel_slot = dloc - lo_per_group[:, None]  # [2*n_tiles, P]
        s8 = np.zeros((2 * n_tiles, P, SW), dtype=float8_e4m3)
        gi, pi = np.nonzero((rel_slot >= 0) & (rel_slot < SW))
        s8[gi, pi, rel_slot[gi, pi].astype(np.int64)] = 1.0
        # -> [p, t, j, w]
        m["s8"] = np.ascontiguousarray(
            s8.reshape(n_tiles, 2, P, SW).transpose(2, 0, 1, 3))
        in_maps.append(m)

    return in_maps, tiles_per_window, ranges, npc, n_win, node_rows_all


def kernel(h, edge_index, edge_rel_emb, W1, b1, W2, b2, num_nodes):
    h = np.ascontiguousarray(h, dtype=np.float32)
    rel = np.ascontiguousarray(edge_rel_emb, dtype=np.float32)
    W1 = np.ascontiguousarray(W1, dtype=np.float32)
    W2 = np.ascontiguousarray(W2, dtype=np.float32)
    b1 = np.ascontiguousarray(b1, dtype=np.float32)
    b2 = np.ascontiguousarray(b2, dtype=np.float32)
    n_nodes = int(num_nodes)
    src = np.asarray(edge_index[0]).astype(np.int64)
    dst = np.asarray(edge_index[1]).astype(np.int64)
    assert n_nodes % N_CORES == 0
    assert h.shape == (n_nodes, D)

    (in_maps, tiles_per_window, ranges, npc, n_win,
     node_rows_all) = _prepare_shards(h, src, dst, rel, n_nodes)

    has_b1 = bool(np.any(b1))
    has_b2 = bool(np.any(b2))
    key = (n_nodes, tuple(tiles_per_window), ranges, has_b1, has_b2)
    if key not in _CACHE:
        _CACHE[key] = _build_program(n_nodes, tiles_per_window, ranges,
                                     has_b1, has_b2)
    nc = _CACHE[key]

    w1_r = np.ascontiguousarray(
        (W1 * W1_SCALE).reshape(2, P, D).transpose(1, 0, 2)).astype(
            float8_e4m3)
    w2_r = np.ascontiguousarray(
        W2.reshape(2, P, D).transpose(1, 0, 2)).astype(bfloat16)
    for m in in_maps:
        m["w1"] = w1_r
        m["w2"] = w2_r
        m["b1r"] = (b1 * W1_SCALE).reshape(1, D).astype(bfloat16)
        m["b2r"] = b2.reshape(1, D).astype(bfloat16)

    trace_kwargs = {}
    if TRACE:
        trace_kwargs = dict(trace=True, tmpdir=TRACE_DIR,
                            trace_cores=list(range(N_CORES)))
    res = run_bass_kernel_spmd(nc, in_maps, core_ids=list(range(N_CORES)),
                               **trace_kwargs)
    out = np.concatenate(
        [np.asarray(res.results[c]["out"])[node_rows_all[c]]
         for c in range(N_CORES)], axis=0)

    # stash for test harnesses
    kernel.last_results = res
    return out.astype(np.float32)


# revision 12
# speedup vs baseline: 1.4378x; 1.2698x over previous
"""BellmanFord GNN message-passing layer on 8 Trainium2 NeuronCores.

Reference computation (all f32):
    x   = h[src] + edge_rel_emb          # [E, D] gather
    hid = relu(x @ W1 + b1)              # [E, D]
    msg = hid @ W2 + b2                  # [E, D]
    agg = segment_sum(msg, dst, N)       # [N, D]
    out = h + agg

Key algebraic restructuring: since W2 is shared across edges, the second
matmul commutes with the scatter:

    segment_sum(hid @ W2 + b2, dst) = segment_sum(hid, dst) @ W2 + deg*b2

so the per-edge work is only m1 (+relu) and the segment-sum; the W2 matmul
runs once per 128-node window instead of once per edge.

v3 (fp8 + balanced engines):
  - m1 in fp8e4m3 with perf_mode=DoubleRow: the PE virtualizes to 256
    contraction rows, so each 128-edge group's full 256-dim contraction is
    ONE matmul (2/tile instead of 4 bf16; ~109ns each either way).  W1 is
    host-scaled by 16 to keep its entries out of fp8-subnormal range; the
    relu evacuation applies scale=1/16 to undo it.
  - hid is evacuated as fp8 in PAIRED 2-tile ops (1024 cols) alternating
    ScalarE/DVE -- evacuation is PSUM-read-bound (~1ns/col) and is the
    second wall after the PE, so per-op overhead is halved and the load
    split evenly.
  - The scatter's 0/1 S matrix ships pre-built from the host (group slot
    ranges are <=9 wide, padded to 16): one 655KB DMA at kernel start,
    zero per-tile DVE work.  Scatter stationaries are fp8 -> FWL weight
    loads at 4 elem/cycle.
  - A ~3.5us PE warmup burst of dummy matmuls at kernel start trips the
    HAM activity monitor out of its cold 1.2GHz state before real work
    arrives (saves >10us of half-clock execution).
  - xq pair DMAs alternate between the sync and gpsimd rings.

Sharding: edges by destination node range (1250 nodes per core) so each
core owns its output slice outright -- no cross-core reduction.  Host
gathers h[src]+rel and transposes into DoubleRow-interleaved fp8 tiles.
"""

import sys

sys.path.insert(0, "/opt/trn_rl_repo")

import numpy as np
from ml_dtypes import bfloat16, float8_e4m3

import concourse.bass as bass
import concourse.mybir as mybir
import concourse.tile as tile
from concourse import bacc
from concourse.bass_utils import run_bass_kernel_spmd

P = 128
D = 256
N_CORES = 8
ET = 256  # edges per macrotile (2 x P)
WIN = P  # nodes per scatter window
SW = 16  # hosted S width (max scatter group slot range, padded)
F32 = mybir.dt.float32
BF16 = mybir.dt.bfloat16
FP8 = mybir.dt.float8e4
AF = mybir.ActivationFunctionType
DR = mybir.MatmulPerfMode.DoubleRow
W1_SCALE = 16.0  # host premultiplies W1 so fp8 entries stay normal

_CACHE = {}
TRACE = False
TRACE_DIR = "/tmp/ktrace"


def _build_program(n_nodes, tiles_per_window, ranges, has_b1, has_b2):
    """Build the SPMD Bass program. Identical for all 8 cores."""
    npc = n_nodes // N_CORES  # nodes per core
    n_win = len(tiles_per_window)
    n_tiles = int(sum(tiles_per_window))
    n_pairs = (n_tiles + 1) // 2
    npc_pad = n_win * WIN

    nc = bacc.Bacc("TRN2", target_bir_lowering=False, debug=False,
                   num_devices=N_CORES)

    hs_d = nc.dram_tensor("h_slice", [npc_pad, D], F32,
                          kind="ExternalInput").ap()
    # xT tiles, pair-packed, DoubleRow-interleaved fp8:
    # xq[pair, p, g, ko, e] = x[(2*pair+g)*ET + e, ko*P + p]
    xq_d = nc.dram_tensor("xq", [n_pairs, P, 2, 2, ET], FP8,
                          kind="ExternalInput").ap()
    # hosted S: s8[p, t, j, w] = [slot(edge (t,j,p)) - lo_tj == w]
    s8_d = nc.dram_tensor("s8", [P, n_tiles, 2, SW], FP8,
                          kind="ExternalInput").ap()
    w1_d = nc.dram_tensor("w1", [P, 2, D], FP8, kind="ExternalInput").ap()
    w2_d = nc.dram_tensor("w2", [P, 2, D], BF16, kind="ExternalInput").ap()
    b1_d = nc.dram_tensor("b1r", [1, D], BF16, kind="ExternalInput").ap()
    b2_d = nc.dram_tensor("b2r", [1, D], BF16, kind="ExternalInput").ap()
    deg_d = nc.dram_tensor("deg", [1, npc_pad], BF16,
                           kind="ExternalInput").ap()
    out_d = nc.dram_tensor("out", [npc_pad, D], F32, kind="ExternalOutput").ap()

    with tile.TileContext(nc) as tc:
        with (
            tc.tile_pool(name="consts", bufs=1) as cb,
            tc.tile_pool(name="x", bufs=9) as x_pool,
            tc.tile_pool(name="hid", bufs=4) as hid_pool,
            tc.tile_pool(name="HT", bufs=2) as ht_pool,
            tc.tile_pool(name="hw", bufs=3) as h_pool,
            tc.tile_pool(name="outw", bufs=3) as out_pool,
            tc.tile_pool(name="psH", bufs=2, space="PSUM") as ps_h,  # hid
            tc.tile_pool(name="psT", bufs=2, space="PSUM") as ps_t,  # HT acc
            tc.tile_pool(name="psA", bufs=1, space="PSUM") as ps_a,  # agg
        ):
            # ---- warmup fodder (no DMA dependencies) ----
            wdum = cb.tile([P, D], BF16)
            nc.vector.memset(wdum[:], 0.0)

            # consts on the gpsimd DMA ring; w1 first (m1 needs it first),
            # then the one-shot S image
            w1_sb = cb.tile([P, 2, D], FP8)
            nc.gpsimd.dma_start(w1_sb[:], w1_d)
            s_all = cb.tile([P, n_tiles, 2, SW], FP8)
            nc.gpsimd.dma_start(s_all[:], s8_d)
            w2_sb = cb.tile([P, 2, D], BF16)
            nc.gpsimd.dma_start(w2_sb[:], w2_d)
            b1_sb = cb.tile([1, D], BF16)
            nc.gpsimd.dma_start(b1_sb[:], b1_d)
            b2_sb = cb.tile([1, D], BF16)
            nc.gpsimd.dma_start(b2_sb[:], b2_d)
            deg_sb = cb.tile([1, npc_pad], BF16)
            nc.gpsimd.dma_start(deg_sb[:], deg_d)
            if has_b1:
                ones_sb = cb.tile([1, P], BF16)
                nc.vector.memset(ones_sb[:], 1.0)

            # ---- PE warmup burst: ~3.5us of dummy matmuls at cold clock
            # trips HAM to 2.4GHz before the first real m1.  Writes go to
            # the agg bank (psA) which the real pipeline only claims from
            # superstep ~26, so no clash with the hid pair banks. ----
            warm_ps = ps_a.tile([P, D], F32, name="agg_ps")
            for i in range(16):
                nc.tensor.matmul(warm_ps[:], lhsT=wdum[:, 0:P],
                                 rhs=wdum[:], start=True, stop=True)

            # ---- software-pipelined emission ----
            tile_win = []
            for w in range(n_win):
                tile_win += [w] * tiles_per_window[w]
            win_first = {}
            win_last = {}
            for ti, w in enumerate(tile_win):
                win_first.setdefault(w, ti)
                win_last[w] = ti
            # last non-empty (tile, j) scatter group per window, for the
            # accumulation-group stop flag
            win_lastg = {}
            for ti, w in enumerate(tile_win):
                for j in range(2):
                    if ranges[2 * ti + j][0] < ranges[2 * ti + j][1]:
                        win_lastg[w] = (ti, j)
            T = n_tiles
            st = {}  # per-tile live tiles
            pair_state = {}
            win_state = {}

            def s_dma(t):
                if t % 2 != 0:
                    return
                x_sb = x_pool.tile([P, 2, 2, ET], FP8, name="x_sb")
                ring = nc.sync if (t // 2) % 2 == 0 else nc.gpsimd
                ring.dma_start(x_sb[:], xq_d[t // 2])
                st[t] = {"x": x_sb, "g": 0}
                if t + 1 < T:
                    st[t + 1] = {"x": x_sb, "g": 1}

            def s_m1(t):
                x_sb = st[t]["x"]
                g = st[t]["g"]
                if t % 2 == 0:
                    hid_ps = ps_h.tile([P, 2, 2, D], F32, name="hid_ps")
                    pair_state[t // 2] = hid_ps
                hid_ps = pair_state[t // 2]
                a = t % 2
                for j in range(2):
                    # one DoubleRow matmul: full 256-dim contraction
                    nc.tensor.matmul(
                        hid_ps[:, a, j],
                        lhsT=x_sb[:, g, :, j * P:(j + 1) * P],
                        rhs=w1_sb[:],
                        start=True,
                        stop=not has_b1,
                        perf_mode=DR,
                    )
                    if has_b1:
                        nc.tensor.matmul(
                            hid_ps[:, a, j],
                            lhsT=ones_sb[:],
                            rhs=b1_sb[:],
                            start=False,
                            stop=True,
                        )

            def s_relu(t):
                # evacuate the (t-1, t) pair in one 1024-col op; scale=1/16
                # undoes the host-side W1 premultiply
                if t % 2 == 0 and t != T - 1:
                    return
                pr = t // 2
                hid_ps = pair_state.pop(pr)
                hid_sb = hid_pool.tile([P, 2, 2, D], FP8, name="hid_sb")
                if pr % 2 == 0:
                    nc.scalar.activation(hid_sb[:], hid_ps[:], AF.Relu,
                                         scale=1.0 / W1_SCALE)
                else:
                    nc.vector.tensor_scalar(
                        out=hid_sb[:], in0=hid_ps[:],
                        scalar1=1.0 / W1_SCALE, scalar2=0.0,
                        op0=mybir.AluOpType.mult,
                        op1=mybir.AluOpType.max)
                st[t]["hidp"] = hid_sb
                if t % 2 == 1:
                    st[t - 1]["hidp"] = hid_sb

            def s_wopen(t):
                w = tile_win[t]
                if t != win_first[w]:
                    return
                ht_ps = ps_t.tile([P, 2, P], F32, name="ht_ps")
                nc.vector.memset(ht_ps[:], 0.0)
                h_sb = h_pool.tile([P, D], F32, name="h_sb")
                nc.gpsimd.dma_start(h_sb[:], hs_d[w * WIN:(w + 1) * WIN])
                win_state[w] = (ht_ps, h_sb)

            def s_sc(t):
                w = tile_win[t]
                ht_ps, _ = win_state[w]
                a = t % 2
                hid_sb = st[t]["hidp"]
                for j in range(2):
                    lo, hi = ranges[2 * t + j]
                    if lo >= hi:
                        continue
                    for dc in range(2):
                        nc.tensor.matmul(
                            ht_ps[:, dc, lo:hi],
                            lhsT=hid_sb[:, a, j, dc * P:(dc + 1) * P],
                            rhs=s_all[:, t, j, 0:hi - lo],
                            start=False,
                            stop=(win_lastg[w] == (t, j)),
                            skip_group_check=True,
                        )
                del st[t]

            def s_htcopy(t):
                w = tile_win[t]
                if t != win_last[w]:
                    return
                ht_ps, h_sb = win_state[w]
                ht_sb = ht_pool.tile([P, 2, P], BF16, name="ht_sb")
                # alternate engines so the copy doesn't queue behind this
                # window's relu ops on one engine
                if w % 2 == 0:
                    nc.scalar.copy(ht_sb[:], ht_ps[:])
                else:
                    nc.vector.tensor_copy(ht_sb[:], ht_ps[:])
                win_state[w] = (ht_sb, h_sb)

            def s_fin(t):
                w = tile_win[t]
                if t != win_last[w]:
                    return
                ht_sb, h_sb = win_state[w]
                agg_ps = ps_a.tile([P, D], F32, name="agg_ps")
                for dc in range(2):
                    nc.tensor.matmul(
                        agg_ps[:],
                        lhsT=ht_sb[:, dc],
                        rhs=w2_sb[:, dc],
                        start=(dc == 0),
                        stop=(dc == 1 and not has_b2),
                    )
                if has_b2:
                    nc.tensor.matmul(
                        agg_ps[:],
                        lhsT=deg_sb[:, w * WIN:(w + 1) * WIN],
                        rhs=b2_sb[:],
                        start=False,
                        stop=True,
                    )
                out_sb = out_pool.tile([P, D], F32, name="out_sb")
                nc.vector.tensor_add(out_sb[:], agg_ps[:], h_sb[:])
                nc.sync.dma_start(out_d[w * WIN:(w + 1) * WIN], out_sb[:])
                del win_state[w]

            # (stage, lag): emission order within a step puts the oldest
            # tile's PE work first so the PE never head-of-line blocks on
            # freshly produced inputs.  PE *stationary* operands (m1's xT,
            # the scatter's hid, m2's HT) must be >= 2 super-steps old when
            # loaded: the PE silicon pulls LDWEIGHTS ahead of in-flight
            # MATMULs, bypassing the semaphore that gates the MATMUL, so a
            # freshly written weight tile can be read stale.
            stages = [(s_fin, 14), (s_htcopy, 11), (s_sc, 9), (s_wopen, 6),
                      (s_relu, 4), (s_m1, 3), (s_dma, -5)]
            L = 1 + max(lag for _, lag in stages)
            lag0 = min(lag for _, lag in stages)
            # first s_fin superstep: ballast into the psA bank must stop
            # before the real agg claims it
            fin0 = min(t for t in range(T)
                       if t == win_last[tile_win[t]]) + 14
            for i in range(lag0, T + L - 1):
                # ballast: keep the PE dense through the pipeline-fill phase
                # so the HAM activity monitor never re-throttles the clock
                if 0 <= i < fin0 - 2:
                    nc.tensor.matmul(warm_ps[:], lhsT=wdum[:, 0:P],
                                     rhs=wdum[:], start=True, stop=True)
                for fn, lag in stages:
                    t_ = i - lag
                    if 0 <= t_ < T:
                        fn(t_)

    nc.compile()
    return nc


def _prepare_shards(h, src, dst, rel, n_nodes):
    """Shard edges by destination range; balance nodes across the per-core
    128-node scatter windows by degree (LPT) so every window's edge count,
    and therefore the shared per-window tile schedule, is near-uniform.

    Returns per-core input arrays, the shared tiles_per_window schedule and
    per-core node->device-row maps for unpermuting the output."""
    npc = n_nodes // N_CORES
    n_win = (npc + WIN - 1) // WIN
    npc_pad = n_win * WIN

    cores = []
    counts = np.zeros((N_CORES, n_win), dtype=np.int64)
    for c in range(N_CORES):
        lo = c * npc
        mask = (dst >= lo) & (dst < lo + npc)
        idx = np.nonzero(mask)[0]
        d_c = (dst[idx] - lo).astype(np.int64)
        deg_cnt = np.bincount(d_c, minlength=npc)
        # LPT: nodes by degree desc -> least-loaded window with free slots
        node_win = np.zeros(npc, dtype=np.int64)
        node_slot = np.zeros(npc, dtype=np.int64)
        win_load = np.zeros(n_win, dtype=np.int64)
        win_fill = np.zeros(n_win, dtype=np.int64)
        for nd in np.argsort(-deg_cnt, kind="stable"):
            open_w = np.nonzero(win_fill < WIN)[0]
            wsel = open_w[np.argmin(win_load[open_w])]
            node_win[nd] = wsel
            node_slot[nd] = win_fill[wsel]
            win_fill[wsel] += 1
            win_load[wsel] += deg_cnt[nd]
        counts[c] = win_load
        cores.append((idx, d_c, node_win, node_slot, deg_cnt))

    tiles_per_window = [
        max(1, int(-(-counts[:, w].max() // ET))) for w in range(n_win)
    ]
    n_tiles = int(sum(tiles_per_window))
    n_pairs = (n_tiles + 1) // 2
    starts = np.concatenate([[0], np.cumsum(tiles_per_window)])

    # Per-(tile, j-group) node-slot ranges: edges are sorted by slot within
    # a window, so each 128-edge group touches only a short contiguous slot
    # range.  The SPMD program is shared, so take the union across cores.
    rng_lo = np.full((n_tiles, 2), P, dtype=np.int64)
    rng_hi = np.zeros((n_tiles, 2), dtype=np.int64)

    percore = []
    node_rows_all = []
    for c in range(N_CORES):
        idx, d_c, node_win, node_slot, deg_cnt = cores[c]
        e_win = node_win[d_c]
        order = np.argsort(e_win, kind="stable")
        idx = idx[order]
        d_c = d_c[order]
        e_win = e_win[order]

        src_pad = np.zeros(n_tiles * ET, dtype=np.int64)
        dloc_pad = np.full(n_tiles * ET, -1.0, dtype=np.float32)
        rel_rows = np.zeros((n_tiles * ET, D), dtype=np.float32)
        real = np.zeros(n_tiles * ET, dtype=bool)
        bounds = np.searchsorted(e_win, np.arange(n_win + 1))
        for w in range(n_win):
            a, b = bounds[w], bounds[w + 1]
            k = b - a
            off = int(starts[w]) * ET
            # sort this window's edges by slot so tile j-groups cover
            # contiguous slot ranges
            slots = node_slot[d_c[a:b]]
            sord = np.argsort(slots, kind="stable")
            src_pad[off:off + k] = src[idx[a:b]][sord]
            dloc_pad[off:off + k] = slots[sord].astype(np.float32)
            rel_rows[off:off + k] = rel[idx[a:b]][sord]
            real[off:off + k] = True
            ss = slots[sord]
            for t in range(tiles_per_window[w]):
                gt = int(starts[w]) + t
                for j in range(2):
                    g0 = t * ET + j * P
                    g1 = min(g0 + P, k)
                    if g0 >= k:
                        continue
                    rng_lo[gt, j] = min(rng_lo[gt, j], int(ss[g0]))
                    rng_hi[gt, j] = max(rng_hi[gt, j], int(ss[g1 - 1]) + 1)

        x_pad = h[src_pad].astype(np.float32)
        x_pad += rel_rows
        x_pad[~real] = 0.0
        x8 = x_pad.astype(float8_e4m3)
        if n_pairs * 2 * ET > x8.shape[0]:
            x8 = np.concatenate(
                [x8, np.zeros((n_pairs * 2 * ET - x8.shape[0], D),
                              dtype=float8_e4m3)], axis=0)
        # [pair, p, g, ko, e]: DoubleRow-interleaved (k = ko*128 + ki)
        xq = np.ascontiguousarray(
            x8.reshape(n_pairs, 2, ET, 2, P).transpose(0, 4, 1, 3, 2))

        node_rows = node_win * WIN + node_slot  # device row of local node
        h_slice = np.zeros((npc_pad, D), dtype=np.float32)
        h_slice[node_rows] = h[c * npc:(c + 1) * npc]
        deg = np.zeros(npc_pad, dtype=np.float32)
        deg[node_rows] = deg_cnt

        percore.append({
            "h_slice": h_slice,
            "xq": xq,
            "dloc_pad": dloc_pad,
            "deg": deg.astype(bfloat16).reshape(1, npc_pad),
        })
        node_rows_all.append(node_rows)

    ranges = tuple(
        (int(min(rng_lo[t, j], rng_hi[t, j])), int(rng_hi[t, j]))
        for t in range(n_tiles) for j in range(2))
    wmax = max(r[1] - r[0] for r in
               [(ranges[g][0], ranges[g][1]) for g in range(2 * n_tiles)])
    assert wmax <= SW, f"scatter group width {wmax} exceeds hosted S {SW}"

    # hosted S image: s8[p, t, j, w] = [slot(edge (t,j,p)) - lo_tj == w]
    lo_per_group = np.array([ranges[g][0] for g in range(2 * n_tiles)],
                            dtype=np.float32)
    in_maps = []
    for c in range(N_CORES):
        m = percore[c]
        dloc = m.pop("dloc_pad").reshape(2 * n_tiles, P)
        rel_slot = dloc - lo_per_group[:, None]  # [2*n_tiles, P]
        s8 = np.zeros((2 * n_tiles, P, SW), dtype=float8_e4m3)
        gi, pi = np.nonzero((rel_slot >= 0) & (rel_slot < SW))
        s8[gi, pi, rel_slot[gi, pi].astype(np.int64)] = 1.0
        # -> [p, t, j, w]
        m["s8"] = np.ascontiguousarray(
            s8.reshape(n_tiles, 2, P, SW).transpose(2, 0, 1, 3))
        in_maps.append(m)

    return in_maps, tiles_per_window, ranges, npc, n_win, node_rows_all


def kernel(h, edge_index, edge_rel_emb, W1, b1, W2, b2, num_nodes):
    h = np.ascontiguousarray(h, dtype=np.float32)
    rel = np.ascontiguousarray(edge_rel_emb, dtype=np.float32)
    W1 = np.ascontiguousarray(W1, dtype=np.float32)
    W2 = np.ascontiguousarray(W2, dtype=np.float32)
    b1 = np.ascontiguousarray(b1, dtype=np.float32)
    b2 = np.ascontiguousarray(b2, dtype=np.float32)
    n_nodes = int(num_nodes)
    src = np.asarray(edge_index[0]).astype(np.int64)
    dst = np.asarray(edge_index[1]).astype(np.int64)
    assert n_nodes % N_CORES == 0
    assert h.shape == (n_nodes, D)

    (in_maps, tiles_per_window, ranges, npc, n_win,
     node_rows_all) = _prepare_shards(h, src, dst, rel, n_nodes)

    has_b1 = bool(np.any(b1))
    has_b2 = bool(np.any(b2))
    key = (n_nodes, tuple(tiles_per_window), ranges, has_b1, has_b2)
    if key not in _CACHE:
        _CACHE[key] = _build_program(n_nodes, tiles_per_window, ranges,
                                     has_b1, has_b2)
    nc = _CACHE[key]

    w1_r = np.ascontiguousarray(
        (W1 * W1_SCALE).reshape(2, P, D).transpose(1, 0, 2)).astype(
            float8_e4m3)
    w2_r = np.ascontiguousarray(
        W2.reshape(2, P, D).transpose(1, 0, 2)).astype(bfloat16)
    for m in in_maps:
        m["w1"] = w1_r
        m["w2"] = w2_r
        m["b1r"] = (b1 * W1_SCALE).reshape(1, D).astype(bfloat16)
        m["b2r"] = b2.reshape(1, D).astype(bfloat16)

    trace_kwargs = {}
    if TRACE:
        trace_kwargs = dict(trace=True, tmpdir=TRACE_DIR,
                            trace_cores=list(range(N_CORES)))
    res = run_bass_kernel_spmd(nc, in_maps, core_ids=list(range(N_CORES)),
                               **trace_kwargs)
    out = np.concatenate(
        [np.asarray(res.results[c]["out"])[node_rows_all[c]]
         for c in range(N_CORES)], axis=0)

    # stash for test harnesses
    kernel.last_results = res
    return out.astype(np.float32)


# revision 15
# speedup vs baseline: 1.6558x; 1.1516x over previous
"""BellmanFord GNN message-passing layer on 8 Trainium2 NeuronCores.

Reference computation (all f32):
    x   = h[src] + edge_rel_emb          # [E, D] gather
    hid = relu(x @ W1 + b1)              # [E, D]
    msg = hid @ W2 + b2                  # [E, D]
    agg = segment_sum(msg, dst, N)       # [N, D]
    out = h + agg

Key algebraic restructuring: since W2 is shared across edges, the second
matmul commutes with the scatter:

    segment_sum(hid @ W2 + b2, dst) = segment_sum(hid, dst) @ W2 + deg*b2

so the per-edge work is only m1 (+relu) and the segment-sum; the W2 matmul
runs once per 128-node window instead of once per edge.

v3 (fp8 + balanced engines):
  - m1 in fp8e4m3 with perf_mode=DoubleRow: the PE virtualizes to 256
    contraction rows, so each 128-edge group's full 256-dim contraction is
    ONE matmul (2/tile instead of 4 bf16; ~109ns each either way).  W1 is
    host-scaled by 16 to keep its entries out of fp8-subnormal range; the
    relu evacuation applies scale=1/16 to undo it.
  - hid is evacuated as fp8 in PAIRED 2-tile ops (1024 cols) alternating
    ScalarE/DVE -- evacuation is PSUM-read-bound (~1ns/col) and is the
    second wall after the PE, so per-op overhead is halved and the load
    split evenly.
  - The scatter's 0/1 S matrix ships pre-built from the host (group slot
    ranges are <=9 wide, padded to 16): one 655KB DMA at kernel start,
    zero per-tile DVE work.  Scatter stationaries are fp8 -> FWL weight
    loads at 4 elem/cycle.
  - A ~3.5us PE warmup burst of dummy matmuls at kernel start trips the
    HAM activity monitor out of its cold 1.2GHz state before real work
    arrives (saves >10us of half-clock execution).
  - xq pair DMAs alternate between the sync and gpsimd rings.

Sharding: edges by destination node range (1250 nodes per core) so each
core owns its output slice outright -- no cross-core reduction.  Host
gathers h[src]+rel and transposes into DoubleRow-interleaved fp8 tiles.
"""

import sys

sys.path.insert(0, "/opt/trn_rl_repo")

import numpy as np
from ml_dtypes import bfloat16, float8_e4m3

import concourse.bass as bass
import concourse.mybir as mybir
import concourse.tile as tile
from concourse import bacc
from concourse.bass_utils import run_bass_kernel_spmd

P = 128
D = 256
N_CORES = 8
ET = 256  # edges per macrotile (2 x P)
WIN = P  # nodes per scatter window
SW = 16  # hosted S width (max scatter group slot range, padded)
F32 = mybir.dt.float32
BF16 = mybir.dt.bfloat16
FP8 = mybir.dt.float8e4
AF = mybir.ActivationFunctionType
DR = mybir.MatmulPerfMode.DoubleRow
W1_SCALE = 16.0  # host premultiplies W1 so fp8 entries stay normal

_CACHE = {}
TRACE = False
TRACE_DIR = "/tmp/ktrace"


def _build_program(n_nodes, tiles_per_window, ranges, has_b1, has_b2):
    """Build the SPMD Bass program. Identical for all 8 cores."""
    npc = n_nodes // N_CORES  # nodes per core
    n_win = len(tiles_per_window)
    n_tiles = int(sum(tiles_per_window))
    n_pairs = (n_tiles + 1) // 2
    npc_pad = n_win * WIN

    nc = bacc.Bacc("TRN2", target_bir_lowering=False, debug=False,
                   num_devices=N_CORES)

    hs_d = nc.dram_tensor("h_slice", [npc_pad, D], F32,
                          kind="ExternalInput").ap()
    # xT tiles, pair-packed, DoubleRow-interleaved fp8:
    # xq[pair, p, g, ko, e] = x[(2*pair+g)*ET + e, ko*P + p]
    xq_d = nc.dram_tensor("xq", [n_pairs, P, 2, 2, ET], FP8,
                          kind="ExternalInput").ap()
    # hosted S: s8[p, t, j, w] = [slot(edge (t,j,p)) - lo_tj == w]
    s8_d = nc.dram_tensor("s8", [P, n_tiles, 2, SW], FP8,
                          kind="ExternalInput").ap()
    w1_d = nc.dram_tensor("w1", [P, 2, D], FP8, kind="ExternalInput").ap()
    w2_d = nc.dram_tensor("w2", [P, 2, D], BF16, kind="ExternalInput").ap()
    b1_d = nc.dram_tensor("b1r", [1, D], BF16, kind="ExternalInput").ap()
    b2_d = nc.dram_tensor("b2r", [1, D], BF16, kind="ExternalInput").ap()
    deg_d = nc.dram_tensor("deg", [1, npc_pad], BF16,
                           kind="ExternalInput").ap()
    out_d = nc.dram_tensor("out", [npc_pad, D], F32, kind="ExternalOutput").ap()

    with tile.TileContext(nc) as tc:
        with (
            tc.tile_pool(name="consts", bufs=1) as cb,
            tc.tile_pool(name="x", bufs=9) as x_pool,
            tc.tile_pool(name="hid", bufs=6) as hid_pool,
            tc.tile_pool(name="HT", bufs=2) as ht_pool,
            tc.tile_pool(name="hw", bufs=3) as h_pool,
            tc.tile_pool(name="outw", bufs=3) as out_pool,
            tc.tile_pool(name="psH", bufs=4, space="PSUM") as ps_h,  # hid
            tc.tile_pool(name="psT", bufs=2, space="PSUM") as ps_t,  # HT acc
            tc.tile_pool(name="psA", bufs=1, space="PSUM") as ps_a,  # agg
        ):
            # ---- warmup fodder (no DMA dependencies) ----
            wdum = cb.tile([P, D], BF16)
            nc.vector.memset(wdum[:], 0.0)

            # consts on the gpsimd DMA ring; w1 first (m1 needs it first),
            # then the one-shot S image
            w1_sb = cb.tile([P, 2, D], FP8)
            nc.gpsimd.dma_start(w1_sb[:], w1_d)
            s_all = cb.tile([P, n_tiles, 2, SW], FP8)
            nc.gpsimd.dma_start(s_all[:], s8_d)
            w2_sb = cb.tile([P, 2, D], BF16)
            nc.gpsimd.dma_start(w2_sb[:], w2_d)
            b1_sb = cb.tile([1, D], BF16)
            nc.gpsimd.dma_start(b1_sb[:], b1_d)
            b2_sb = cb.tile([1, D], BF16)
            nc.gpsimd.dma_start(b2_sb[:], b2_d)
            deg_sb = cb.tile([1, npc_pad], BF16)
            nc.gpsimd.dma_start(deg_sb[:], deg_d)
            if has_b1:
                ones_sb = cb.tile([1, P], BF16)
                nc.vector.memset(ones_sb[:], 1.0)

            # ---- PE warmup burst: ~3.5us of dummy matmuls at cold clock
            # trips HAM to 2.4GHz before the first real m1.  Writes go to
            # the agg bank (psA) which the real pipeline only claims from
            # superstep ~26, so no clash with the hid pair banks. ----
            warm_ps = ps_a.tile([P, D], F32, name="agg_ps")
            for i in range(16):
                nc.tensor.matmul(warm_ps[:], lhsT=wdum[:, 0:P],
                                 rhs=wdum[:], start=True, stop=True)

            # ---- software-pipelined emission ----
            tile_win = []
            for w in range(n_win):
                tile_win += [w] * tiles_per_window[w]
            win_first = {}
            win_last = {}
            for ti, w in enumerate(tile_win):
                win_first.setdefault(w, ti)
                win_last[w] = ti
            # last non-empty (tile, j) scatter group per window, for the
            # accumulation-group stop flag
            win_lastg = {}
            for ti, w in enumerate(tile_win):
                for j in range(2):
                    if ranges[2 * ti + j][0] < ranges[2 * ti + j][1]:
                        win_lastg[w] = (ti, j)
            T = n_tiles
            st = {}  # per-tile live tiles
            pair_state = {}
            win_state = {}

            def s_dma(t):
                if t % 2 != 0:
                    return
                x_sb = x_pool.tile([P, 2, 2, ET], FP8, name="x_sb")
                ring = nc.sync if (t // 2) % 2 == 0 else nc.gpsimd
                ring.dma_start(x_sb[:], xq_d[t // 2])
                st[t] = {"x": x_sb, "g": 0}
                if t + 1 < T:
                    st[t + 1] = {"x": x_sb, "g": 1}

            def s_m1(t):
                x_sb = st[t]["x"]
                g = st[t]["g"]
                hid_ps = ps_h.tile([P, 2, D], F32, name="hid_ps")
                st[t]["ps"] = hid_ps
                for j in range(2):
                    # one DoubleRow matmul: full 256-dim contraction
                    nc.tensor.matmul(
                        hid_ps[:, j],
                        lhsT=x_sb[:, g, :, j * P:(j + 1) * P],
                        rhs=w1_sb[:],
                        start=True,
                        stop=not has_b1,
                        perf_mode=DR,
                    )
                    if has_b1:
                        nc.tensor.matmul(
                            hid_ps[:, j],
                            lhsT=ones_sb[:],
                            rhs=b1_sb[:],
                            start=False,
                            stop=True,
                        )

            def s_relu(t):
                # evacuate hid; scale=1/16 undoes the host W1 premultiply
                hid_ps = st[t].pop("ps")
                hid_sb = hid_pool.tile([P, 2, D], FP8, name="hid_sb")
                if t % 2 == 0:
                    nc.scalar.activation(hid_sb[:], hid_ps[:], AF.Relu,
                                         scale=1.0 / W1_SCALE)
                else:
                    nc.vector.tensor_scalar(
                        out=hid_sb[:], in0=hid_ps[:],
                        scalar1=1.0 / W1_SCALE, scalar2=0.0,
                        op0=mybir.AluOpType.mult,
                        op1=mybir.AluOpType.max)
                st[t]["hidp"] = hid_sb

            def s_wopen(t):
                w = tile_win[t]
                if t != win_first[w]:
                    return
                ht_ps = ps_t.tile([P, 2, P], F32, name="ht_ps")
                nc.vector.memset(ht_ps[:], 0.0)
                h_sb = h_pool.tile([P, D], F32, name="h_sb")
                nc.gpsimd.dma_start(h_sb[:], hs_d[w * WIN:(w + 1) * WIN])
                win_state[w] = (ht_ps, h_sb)

            def s_sc(t):
                w = tile_win[t]
                ht_ps, _ = win_state[w]
                hid_sb = st[t]["hidp"]
                for j in range(2):
                    lo, hi = ranges[2 * t + j]
                    if lo >= hi:
                        continue
                    for dc in range(2):
                        nc.tensor.matmul(
                            ht_ps[:, dc, lo:hi],
                            lhsT=hid_sb[:, j, dc * P:(dc + 1) * P],
                            rhs=s_all[:, t, j, 0:hi - lo],
                            start=False,
                            stop=(win_lastg[w] == (t, j)),
                            skip_group_check=True,
                        )
                del st[t]

            def s_htcopy(t):
                w = tile_win[t]
                if t != win_last[w]:
                    return
                ht_ps, h_sb = win_state[w]
                ht_sb = ht_pool.tile([P, 2, P], BF16, name="ht_sb")
                # alternate engines so the copy doesn't queue behind this
                # window's relu ops on one engine
                if w % 2 == 0:
                    nc.scalar.copy(ht_sb[:], ht_ps[:])
                else:
                    nc.vector.tensor_copy(ht_sb[:], ht_ps[:])
                win_state[w] = (ht_sb, h_sb)

            def s_fin(t):
                w = tile_win[t]
                if t != win_last[w]:
                    return
                ht_sb, h_sb = win_state[w]
                agg_ps = ps_a.tile([P, D], F32, name="agg_ps")
                for dc in range(2):
                    nc.tensor.matmul(
                        agg_ps[:],
                        lhsT=ht_sb[:, dc],
                        rhs=w2_sb[:, dc],
                        start=(dc == 0),
                        stop=(dc == 1 and not has_b2),
                    )
                if has_b2:
                    nc.tensor.matmul(
                        agg_ps[:],
                        lhsT=deg_sb[:, w * WIN:(w + 1) * WIN],
                        rhs=b2_sb[:],
                        start=False,
                        stop=True,
                    )
                out_sb = out_pool.tile([P, D], F32, name="out_sb")
                nc.vector.tensor_add(out_sb[:], agg_ps[:], h_sb[:])
                nc.sync.dma_start(out_d[w * WIN:(w + 1) * WIN], out_sb[:])
                del win_state[w]

            # (stage, lag): emission order within a step puts the oldest
            # tile's PE work first so the PE never head-of-line blocks on
            # freshly produced inputs.  PE *stationary* operands (m1's xT,
            # the scatter's hid, m2's HT) must be >= 2 super-steps old when
            # loaded: the PE silicon pulls LDWEIGHTS ahead of in-flight
            # MATMULs, bypassing the semaphore that gates the MATMUL, so a
            # freshly written weight tile can be read stale.
            stages = [(s_fin, 14), (s_htcopy, 11), (s_sc, 9), (s_wopen, 6),
                      (s_relu, 4), (s_m1, 3), (s_dma, -5)]
            L = 1 + max(lag for _, lag in stages)
            lag0 = min(lag for _, lag in stages)
            # first s_fin superstep: ballast into the psA bank must stop
            # before the real agg claims it
            fin0 = min(t for t in range(T)
                       if t == win_last[tile_win[t]]) + 14
            for i in range(lag0, T + L - 1):
                # ballast: keep the PE dense through the pipeline-fill phase
                # so the HAM activity monitor never re-throttles the clock
                if 0 <= i < fin0 - 2:
                    nc.tensor.matmul(warm_ps[:], lhsT=wdum[:, 0:P],
                                     rhs=wdum[:], start=True, stop=True)
                for fn, lag in stages:
                    t_ = i - lag
                    if 0 <= t_ < T:
                        fn(t_)

    nc.compile()
    return nc


def _prepare_shards(h, src, dst, rel, n_nodes):
    """Shard edges by destination range; balance nodes across the per-core
    128-node scatter windows by degree (LPT) so every window's edge count,
    and therefore the shared per-window tile schedule, is near-uniform.

    Returns per-core input arrays, the shared tiles_per_window schedule and
    per-core node->device-row maps for unpermuting the output."""
    npc = n_nodes // N_CORES
    n_win = (npc + WIN - 1) // WIN
    npc_pad = n_win * WIN

    cores = []
    counts = np.zeros((N_CORES, n_win), dtype=np.int64)
    for c in range(N_CORES):
        lo = c * npc
        mask = (dst >= lo) & (dst < lo + npc)
        idx = np.nonzero(mask)[0]
        d_c = (dst[idx] - lo).astype(np.int64)
        deg_cnt = np.bincount(d_c, minlength=npc)
        # LPT: nodes by degree desc -> least-loaded window with free slots
        node_win = np.zeros(npc, dtype=np.int64)
        node_slot = np.zeros(npc, dtype=np.int64)
        win_load = np.zeros(n_win, dtype=np.int64)
        win_fill = np.zeros(n_win, dtype=np.int64)
        for nd in np.argsort(-deg_cnt, kind="stable"):
            open_w = np.nonzero(win_fill < WIN)[0]
            wsel = open_w[np.argmin(win_load[open_w])]
            node_win[nd] = wsel
            node_slot[nd] = win_fill[wsel]
            win_fill[wsel] += 1
            win_load[wsel] += deg_cnt[nd]
        counts[c] = win_load
        cores.append((idx, d_c, node_win, node_slot, deg_cnt))

    tiles_per_window = [
        max(1, int(-(-counts[:, w].max() // ET))) for w in range(n_win)
    ]
    n_tiles = int(sum(tiles_per_window))
    n_pairs = (n_tiles + 1) // 2
    starts = np.concatenate([[0], np.cumsum(tiles_per_window)])

    # Per-(tile, j-group) node-slot ranges: edges are sorted by slot within
    # a window, so each 128-edge group touches only a short contiguous slot
    # range.  The SPMD program is shared, so take the union across cores.
    rng_lo = np.full((n_tiles, 2), P, dtype=np.int64)
    rng_hi = np.zeros((n_tiles, 2), dtype=np.int64)

    percore = []
    node_rows_all = []
    for c in range(N_CORES):
        idx, d_c, node_win, node_slot, deg_cnt = cores[c]
        e_win = node_win[d_c]
        order = np.argsort(e_win, kind="stable")
        idx = idx[order]
        d_c = d_c[order]
        e_win = e_win[order]

        src_pad = np.zeros(n_tiles * ET, dtype=np.int64)
        dloc_pad = np.full(n_tiles * ET, -1.0, dtype=np.float32)
        rel_rows = np.zeros((n_tiles * ET, D), dtype=np.float32)
        real = np.zeros(n_tiles * ET, dtype=bool)
        bounds = np.searchsorted(e_win, np.arange(n_win + 1))
        for w in range(n_win):
            a, b = bounds[w], bounds[w + 1]
            k = b - a
            off = int(starts[w]) * ET
            # sort this window's edges by slot so tile j-groups cover
            # contiguous slot ranges
            slots = node_slot[d_c[a:b]]
            sord = np.argsort(slots, kind="stable")
            src_pad[off:off + k] = src[idx[a:b]][sord]
            dloc_pad[off:off + k] = slots[sord].astype(np.float32)
            rel_rows[off:off + k] = rel[idx[a:b]][sord]
            real[off:off + k] = True
            ss = slots[sord]
            for t in range(tiles_per_window[w]):
                gt = int(starts[w]) + t
                for j in range(2):
                    g0 = t * ET + j * P
                    g1 = min(g0 + P, k)
                    if g0 >= k:
                        continue
                    rng_lo[gt, j] = min(rng_lo[gt, j], int(ss[g0]))
                    rng_hi[gt, j] = max(rng_hi[gt, j], int(ss[g1 - 1]) + 1)

        x_pad = h[src_pad].astype(np.float32)
        x_pad += rel_rows
        x_pad[~real] = 0.0
        x8 = x_pad.astype(float8_e4m3)
        if n_pairs * 2 * ET > x8.shape[0]:
            x8 = np.concatenate(
                [x8, np.zeros((n_pairs * 2 * ET - x8.shape[0], D),
                              dtype=float8_e4m3)], axis=0)
        # [pair, p, g, ko, e]: DoubleRow-interleaved (k = ko*128 + ki)
        xq = np.ascontiguousarray(
            x8.reshape(n_pairs, 2, ET, 2, P).transpose(0, 4, 1, 3, 2))

        node_rows = node_win * WIN + node_slot  # device row of local node
        h_slice = np.zeros((npc_pad, D), dtype=np.float32)
        h_slice[node_rows] = h[c * npc:(c + 1) * npc]
        deg = np.zeros(npc_pad, dtype=np.float32)
        deg[node_rows] = deg_cnt

        percore.append({
            "h_slice": h_slice,
            "xq": xq,
            "dloc_pad": dloc_pad,
            "deg": deg.astype(bfloat16).reshape(1, npc_pad),
        })
        node_rows_all.append(node_rows)

    ranges = tuple(
        (int(min(rng_lo[t, j], rng_hi[t, j])), int(rng_hi[t, j]))
        for t in range(n_tiles) for j in range(2))
    wmax = max(r[1] - r[0] for r in
               [(ranges[g][0], ranges[g][1]) for g in range(2 * n_tiles)])
    assert wmax <= SW, f"scatter group width {wmax} exceeds hosted S {SW}"

    # hosted S image: s8[p, t, j, w] = [slot(edge (t,j,p)) - lo_tj == w]
    lo_per_group = np.array([ranges[g][0] for g in range(2 * n_tiles)],
                            dtype=np.float32)
    in_maps = []
    for c in range(N_CORES):
        m = percore[c]
        dloc = m.pop("dloc_pad").reshape(2 * n_tiles, P)
        rel_slot = dloc - lo_per_group[:, None]  # [2*n_tiles, P]
        s8 = np.zeros((2 * n_tiles, P, SW), dtype=float8_e4m3)
        gi, pi = np.nonzero((rel_slot >= 0) & (rel_slot < SW))
        s8[gi, pi, rel_slot[gi, pi].astype(np.int64)] = 1.0
        # -> [p, t, j, w]
        m["s8"] = np.ascontiguousarray(
            s8.reshape(n_tiles, 2, P, SW).transpose(2, 0, 1, 3))
        in_maps.append(m)

    return in_maps, tiles_per_window, ranges, npc, n_win, node_rows_all


def kernel(h, edge_index, edge_rel_emb, W1, b1, W2, b2, num_nodes):
    h = np.ascontiguousarray(h, dtype=np.float32)
    rel = np.ascontiguousarray(edge_rel_emb, dtype=np.float32)
    W1 = np.ascontiguousarray(W1, dtype=np.float32)
    W2 = np.ascontiguousarray(W2, dtype=np.float32)
    b1 = np.ascontiguousarray(b1, dtype=np.float32)
    b2 = np.ascontiguousarray(b2, dtype=np.float32)
    n_nodes = int(num_nodes)
    src = np.asarray(edge_index[0]).astype(np.int64)
    dst = np.asarray(edge_index[1]).astype(np.int64)
    assert n_nodes % N_CORES == 0
    assert h.shape == (n_nodes, D)

    (in_maps, tiles_per_window, ranges, npc, n_win,
     node_rows_all) = _prepare_shards(h, src, dst, rel, n_nodes)

    has_b1 = bool(np.any(b1))
    has_b2 = bool(np.any(b2))
    key = (n_nodes, tuple(tiles_per_window), ranges, has_b1, has_b2)
    if key not in _CACHE:
        _CACHE[key] = _build_program(n_nodes, tiles_per_window, ranges,
                                     has_b1, has_b2)
    nc = _CACHE[key]

    w1_r = np.ascontiguousarray(
        (W1 * W1_SCALE).reshape(2, P, D).transpose(1, 0, 2)).astype(
            float8_e4m3)
    w2_r = np.ascontiguousarray(
        W2.reshape(2, P, D).transpose(1, 0, 2)).astype(bfloat16)
    for m in in_maps:
        m["w1"] = w1_r
        m["w2"] = w2_r
        m["b1r"] = (b1 * W1_SCALE).reshape(1, D).astype(bfloat16)
        m["b2r"] = b2.reshape(1, D).astype(bfloat16)

    trace_kwargs = {}
    if TRACE:
        trace_kwargs = dict(trace=True, tmpdir=TRACE_DIR,
                            trace_cores=list(range(N_CORES)))
    res = run_bass_kernel_spmd(nc, in_maps, core_ids=list(range(N_CORES)),
                               **trace_kwargs)
    out = np.concatenate(
        [np.asarray(res.results[c]["out"])[node_rows_all[c]]
         for c in range(N_CORES)], axis=0)

    # stash for test harnesses
    kernel.last_results = res
    return out.astype(np.float32)
